# revision 14
# baseline (speedup 1.0000x reference)
"""Trainium2 Bass kernel for nn_Attention_52012053954690.

Module: 3D self-attention over [B=4, C=256, 24,24,24] feature maps.
  f = Wf@x                      [B, 32, N]   N = 13824
  g = maxpool2(Wg@x)            [B, 32, M]   M = 1728
  h = maxpool2(Wh@x)            [B, 32, M]
  beta = softmax(f^T g, dim=m)  [B, N, M]
  o = h @ beta^T                [B, 32, N]
  out = gamma * (Wv @ o) + x    [B, 256, N]

Sharding (8 cores, no collectives): core c handles batch c//2, query-half
c%2 (d-slices of 12).  Each core receives x[b] with its d-half rotated to
the front, so queries are always columns 0:6912; softmax over m is
permutation invariant so g/h built from the rotated x are consistent.

Per-core dataflow (everything fp32; matmuls as float32r; exp scores and
hT in bf16):
  scoresT[m_tile, q] = g_tile^T @ f          (K=32, PE)
  expST = exp(scoresT)                       (ScalarE, PSUM->SBUF, bf16)
  o_aug[0:33, q] += hT_aug_tile^T @ expST    (K=128, PE; row 32 = softmax sums
                                              via a ones-column in hT_aug)
  r = 1/sums (reciprocal_approx_fast), broadcast over partitions
  o_norm = o_aug[0:32] * r                   (DVE)
  out = (gamma*Wv)^T^T @ o_norm + x          (PE + DVE residual add)
"""

import os
import sys

sys.path.insert(0, "/opt/trn_rl_repo")

import numpy as np

import concourse.bass as bass
import concourse.mybir as mybir
import concourse.tile as tile
from concourse import bacc
from concourse.bass_utils import run_bass_kernel_spmd

F32 = mybir.dt.float32
F32R = mybir.dt.float32r
BF16 = mybir.dt.bfloat16

P = 128
C = 256          # input channels
CQ = 32          # query channels
D = 24           # spatial extent
NL = 6912        # queries per core (half of 13824)
NF = 13824       # full spatial size
M = 1728         # pooled key/value length
MP = 1792        # m padded to 14*128
NMT = 14         # m tiles of 128
MAXMM = 512      # max matmul free dim (one PSUM bank, fp32)

# q-chunks for the main loop: 8 x 864 (aligned to the 1728-col x tiles)
QN = 864
QCH = [(i * QN, QN) for i in range(8)]
XT = 1728  # x tile width (3 d-slices)


def _subs(n, s=MAXMM):
    """split [0, n) into subchunks of length <= s"""
    return [(o, min(s, n - o)) for o in range(0, n, s)]


def build_nc():
    nc = bacc.Bacc("TRN2", target_bir_lowering=False)

    x_ext = nc.declare_dram_parameter("x", [C, NF], F32R, isOutput=False)
    wf_ext = nc.declare_dram_parameter("wf", [C, CQ], F32R, isOutput=False)
    wg_ext = nc.declare_dram_parameter("wg", [C, CQ], F32R, isOutput=False)
    wh_ext = nc.declare_dram_parameter("wh", [C, CQ], F32R, isOutput=False)
    wv_ext = nc.declare_dram_parameter("wv", [CQ, C], F32R, isOutput=False)
    out_ext = nc.declare_dram_parameter("out", [C, NL], F32, isOutput=True)

    with tile.TileContext(nc) as tc:
        kernel_body(nc, tc, x_ext, wf_ext, wg_ext, wh_ext, wv_ext, out_ext)
    nc.compile()
    return nc


def kernel_body(nc, tc, x_ext, wf_ext, wg_ext, wh_ext, wv_ext, out_ext):
    from contextlib import ExitStack

    with ExitStack() as ctx:
        const = ctx.enter_context(tc.tile_pool(name="const", bufs=1))
        xp = ctx.enter_context(tc.tile_pool(name="xp", bufs=1))
        fgh = ctx.enter_context(tc.tile_pool(name="fgh", bufs=1))
        pool1 = ctx.enter_context(tc.tile_pool(name="pool1", bufs=2))
        pool2 = ctx.enter_context(tc.tile_pool(name="pool2", bufs=2))
        work = ctx.enter_context(tc.tile_pool(name="work", bufs=2))
        nrm = ctx.enter_context(tc.tile_pool(name="nrm", bufs=2))
        # PSUM: sc 2x2 banks + o 1x2 banks + fin 2x1 bank = 8 banks
        ps_sc = ctx.enter_context(tc.tile_pool(name="ps_sc", bufs=2, space="PSUM"))
        ps_o = ctx.enter_context(tc.tile_pool(name="ps_o", bufs=1, space="PSUM"))
        ps_fin = ctx.enter_context(tc.tile_pool(name="ps_fin", bufs=2, space="PSUM"))

        # ---- weights in ----
        wf_sb = const.tile([P, 2 * CQ], F32R)
        wg_sb = const.tile([P, 2 * CQ], F32R)
        wh_sb = const.tile([P, 2 * CQ], F32R)
        wv_sb = const.tile([CQ, C], F32R)
        for w_sb, w_ext in ((wf_sb, wf_ext), (wg_sb, wg_ext), (wh_sb, wh_ext)):
            for k in range(2):
                nc.sync.dma_start(
                    out=w_sb[:, k * CQ:(k + 1) * CQ],
                    in_=w_ext[k * P:(k + 1) * P, :],
                )
        nc.sync.dma_start(out=wv_sb[:, :], in_=wv_ext[:, :])

        eye = const.tile([CQ, CQ], F32)
        from concourse.masks import make_identity
        make_identity(nc, eye[:, :])

        # ---- x in: per (k-chunk, d-slice-of-3) tiles so each consumer
        # depends on exactly one DMA (walrus caps sync waits per matmul) ----
        x_sb = [
            [
                xp.tile([P, XT], F32R, tag=f"x{k}_{ds}", name=f"x_sb{k}_{ds}")
                for ds in range(NF // XT)
            ]
            for k in range(2)
        ]
        for ds in range(NF // XT):
            for k in range(2):
                nc.sync.dma_start(
                    out=x_sb[k][ds][:, :],
                    in_=x_ext[k * P:(k + 1) * P, ds * XT:(ds + 1) * XT],
                )

        def xv(k, c0, cn):
            """view of x columns [c0, c0+cn) -- must lie in one tile"""
            ds, o = divmod(c0, XT)
            assert o + cn <= XT, (c0, cn)
            return x_sb[k][ds][:, o:o + cn]


        # ---- g/h projection + 2x2x2 maxpool ----
        # x spatial layout per core: [d=24, h=24, w=24] flattened; d in two
        # halves of 12 (query half first).  Per d-slice [32, 576]:
        #   proj (PE) -> psum; copy psum->sbuf (ScalarE, idle in prologue);
        #   3 max passes on gpsimd: w-pairs, h-pairs, d-pairs.
        g_sb = fgh.tile([CQ, MP], F32R)
        h_sb = fgh.tile([CQ, MP], F32)
        for t_sb, w_sb in ((g_sb, wg_sb), (h_sb, wh_sb)):
            t2_parts = {}
            for d in range(D):
                # two 288-col halves, each bank-aligned (bank = 512 fp32)
                ps_p = ps_sc.tile([CQ, 2, 512], F32, tag="sc")
                for j in range(2):
                    for k in range(2):
                        nc.tensor.matmul(
                            out=ps_p[:, j, 0:288],
                            lhsT=w_sb[:, k * CQ:(k + 1) * CQ],
                            rhs=xv(k, d * 576 + j * 288, 288),
                            start=(k == 0),
                            stop=(k == 1),
                        )
                pre = pool1.tile([CQ, 576], F32, tag="pre")
                nc.vector.tensor_copy(
                    out=pre[:, :].rearrange("p (j a) -> p j a", j=2),
                    in_=ps_p[:, :, 0:288],
                )
                # pass1: w-pairs [h=24, w=24] -> [24, 12]
                t1 = pool1.tile([CQ, 288], F32, tag="t1")
                prev = pre[:, :].rearrange("p (a b) -> p a b", b=2)
                nc.vector.tensor_tensor(
                    out=t1[:, :],
                    in0=prev[:, :, 0],
                    in1=prev[:, :, 1],
                    op=mybir.AluOpType.max,
                )
                # pass2: h-pairs [24, 12] -> [12, 12]
                t2 = pool2.tile([CQ, 144], F32, tag="t2")
                t1v = t1[:, :].rearrange("p (h2 q w) -> p h2 q w", h2=12, q=2)
                nc.vector.tensor_tensor(
                    out=t2[:, :].rearrange("p (h2 w) -> p h2 w", h2=12),
                    in0=t1v[:, :, 0, :],
                    in1=t1v[:, :, 1, :],
                    op=mybir.AluOpType.max,
                )
                t2_parts[d] = t2
                # pass3: d-pairs
                if d % 2 == 1:
                    nc.vector.tensor_tensor(
                        out=t_sb[:, (d // 2) * 144:(d // 2) * 144 + 144],
                        in0=t2_parts[d - 1][:, :],
                        in1=t2[:, :],
                        op=mybir.AluOpType.max,
                    )
            # pad m 1728:1792 with zeros (exp(0)=1 but excluded from sums).
            # memset can't write float32r; multiply real data by 0 instead.
            nc.vector.tensor_scalar_mul(
                out=t_sb[:, M:MP], in0=t_sb[:, M - (MP - M):M], scalar1=0.0
            )

        # ---- hT_aug [128, 14*64] bf16: transposed h, a ones column at 32,
        # zero pad to 64 (matmul stationary columns must be 32-aligned) ----
        MTW = 2 * CQ  # 64 cols per m-tile
        hT = fgh.tile([P, NMT * MTW], BF16)
        nc.vector.memset(hT[:, :], 0.0)
        for mt in range(NMT):
            ps_t = ps_fin.tile([P, CQ], F32, tag="fin")
            nc.tensor.transpose(
                out=ps_t[:, :],
                in_=h_sb[:, mt * P:(mt + 1) * P],
                identity=eye[:, :],
            )
            c0 = mt * MTW
            nc.vector.tensor_copy(out=hT[:, c0:c0 + CQ], in_=ps_t[:, :])
            if mt < NMT - 1:
                nc.vector.memset(hT[:, c0 + CQ:c0 + CQ + 1], 1.0)
            else:
                # last tile: only m 1664:1728 are real
                nc.vector.memset(hT[0:M - 13 * P, c0 + CQ:c0 + CQ + 1], 1.0)

        # ---- main loop over q-chunks ----
        # persistent r tiles (stream_shuffle reads all 32 partitions; memset
        # once so partitions 1:32 are initialized)
        r_ts = [nrm.tile([CQ, QN], F32, tag=f"r{i}", name=f"r_t{i}") for i in range(2)]
        for r_t in r_ts:
            nc.vector.memset(r_t[:, :], 0.0)
        for qi, (q0, qn) in enumerate(QCH):
            # f projection for this q-chunk only (saves 27.6KB of SBUF)
            ps_fc = ps_sc.tile([CQ, 1024], F32, tag="sc")
            for s0, sn in _subs(qn):
                for k in range(2):
                    nc.tensor.matmul(
                        out=ps_fc[:, s0:s0 + sn],
                        lhsT=wf_sb[:, k * CQ:(k + 1) * CQ],
                        rhs=xv(k, q0 + s0, sn),
                        start=(k == 0),
                        stop=(k == 1),
                    )
            f_c = work.tile([CQ, QN], F32R, tag="fc")
            nc.vector.tensor_copy(out=f_c[:, 0:qn], in_=ps_fc[:, 0:qn])
            ps_oa = ps_o.tile([2 * CQ, QN], F32, tag="o")
            for mt in range(NMT):
                ps_s = ps_sc.tile([P, 1024], F32, tag="sc")
                for s0, sn in _subs(qn):
                    nc.tensor.matmul(
                        out=ps_s[:, s0:s0 + sn],
                        lhsT=g_sb[:, mt * P:(mt + 1) * P],
                        rhs=f_c[:, s0:s0 + sn],
                        start=True,
                        stop=True,
                    )
                est = work.tile([P, QN], BF16, tag="est")
                nc.scalar.activation(
                    out=est[:, 0:qn],
                    in_=ps_s[:, 0:qn],
                    func=mybir.ActivationFunctionType.Exp,
                )
                for s0, sn in _subs(qn):
                    nc.tensor.matmul(
                        out=ps_oa[:, s0:s0 + sn],
                        lhsT=hT[:, mt * MTW:(mt + 1) * MTW],
                        rhs=est[:, s0:s0 + sn],
                        start=(mt == 0),
                        stop=(mt == NMT - 1),
                    )
            # softmax denominators -> broadcast reciprocal.  DVE lanes are
            # partition-locked, so stage the sums row: copy psum[32]->sbuf[32]
            # then DMA it to partition 0, then everything stays aligned.
            srow = nrm.tile([CQ + 1, QN], F32, tag="srow")
            nc.vector.tensor_copy(
                out=srow[CQ:CQ + 1, 0:qn], in_=ps_oa[CQ:CQ + 1, 0:qn]
            )
            nc.sync.dma_start(out=srow[0:1, 0:qn], in_=srow[CQ:CQ + 1, 0:qn])
            r_t = r_ts[qi % 2]
            nc.vector.reciprocal_approx_fast(
                out=r_t[0:1, 0:qn], in_=srow[0:1, 0:qn]
            )
            nc.vector.stream_shuffle(
                out=r_t[:, 0:qn], in_=r_t[:, 0:qn], mask=[0] * 32
            )
            o_n = nrm.tile([CQ, QN], F32R, tag="on")
            nc.vector.tensor_tensor(
                out=o_n[:, 0:qn],
                in0=ps_oa[0:CQ, 0:qn],
                in1=r_t[:, 0:qn],
                op=mybir.AluOpType.mult,
            )
            # final projection + residual + store
            for cc in range(2):
                for s0, sn in _subs(qn):
                    ps_f2 = ps_fin.tile([P, MAXMM], F32, tag="fin")
                    nc.tensor.matmul(
                        out=ps_f2[:, 0:sn],
                        lhsT=wv_sb[:, cc * P:(cc + 1) * P],
                        rhs=o_n[:, s0:s0 + sn],
                        start=True,
                        stop=True,
                    )
                    o_sb = work.tile([P, MAXMM], F32, tag="osb")
                    nc.vector.tensor_tensor(
                        out=o_sb[:, 0:sn],
                        in0=ps_f2[:, 0:sn],
                        in1=xv(cc, q0 + s0, sn),
                        op=mybir.AluOpType.add,
                    )
                    nc.sync.dma_start(
                        out=out_ext[cc * P:(cc + 1) * P, q0 + s0:q0 + s0 + sn],
                        in_=o_sb[:, 0:sn],
                    )


_NC_CACHE = None


def _get_nc():
    global _NC_CACHE
    if _NC_CACHE is None:
        _NC_CACHE = build_nc()
    return _NC_CACHE


def _prep_in_maps(x, Wf, Wg, Wh, Wv, gamma):
    """Build the 8 per-core input maps (host-side shard + permute)."""
    B = x.shape[0]
    x = np.asarray(x, dtype=np.float32).reshape(B, C, D, D * D)
    wvg = (np.float32(gamma) * np.asarray(Wv, dtype=np.float32)).T.copy()
    wfT = np.ascontiguousarray(np.asarray(Wf, np.float32).T)
    wgT = np.ascontiguousarray(np.asarray(Wg, np.float32).T)
    whT = np.ascontiguousarray(np.asarray(Wh, np.float32).T)
    in_maps = []
    for c in range(8):
        b, s = divmod(c, 2)
        xb = x[b]
        if s == 0:
            xp = xb
        else:
            xp = np.concatenate([xb[:, 12:24], xb[:, 0:12]], axis=1)
        in_maps.append({
            "x": np.ascontiguousarray(xp.reshape(C, NF)),
            "wf": wfT, "wg": wgT, "wh": whT, "wv": wvg,
        })
    return in_maps


def run(inputs, trace=False, **kw):
    nc = _get_nc()
    in_maps = _prep_in_maps(**inputs)
    res = run_bass_kernel_spmd(
        nc, in_maps, core_ids=list(range(8)), trace=trace, **kw
    )
    x = np.asarray(inputs["x"], dtype=np.float32)
    B = x.shape[0]
    out = np.empty((B, C, D, D, D), dtype=np.float32)
    for c in range(8):
        b, s = divmod(c, 2)
        out[b, :, 12 * s:12 * (s + 1)] = res.results[c]["out"].reshape(C, 12, D, D)
    return out, res


def kernel(**inputs):
    return run(inputs)[0]


# revision 16
# speedup vs baseline: 1.1669x; 1.1669x over previous
"""Trainium2 Bass kernel for nn_Attention_52012053954690.

Module: 3D self-attention over [B=4, C=256, 24,24,24] feature maps.
  f = Wf@x                      [B, 32, N]   N = 13824
  g = maxpool2(Wg@x)            [B, 32, M]   M = 1728
  h = maxpool2(Wh@x)            [B, 32, M]
  beta = softmax(f^T g, dim=m)  [B, N, M]
  o = h @ beta^T                [B, 32, N]
  out = gamma * (Wv @ o) + x    [B, 256, N]

Sharding (8 cores, no collectives): core c handles batch c//2, query-half
c%2 (d-slices of 12).  Each core receives x[b] with its d-half rotated to
the front, so queries are always columns 0:6912; softmax over m is
permutation invariant so g/h built from the rotated x are consistent.

Per-core dataflow (everything fp32; matmuls as float32r; exp scores and
hT in bf16):
  scoresT[m_tile, q] = g_tile^T @ f          (K=32, PE)
  expST = exp(scoresT)                       (ScalarE, PSUM->SBUF, bf16)
  o_aug[0:33, q] += hT_aug_tile^T @ expST    (K=128, PE; row 32 = softmax sums
                                              via a ones-column in hT_aug)
  r = 1/sums (reciprocal_approx_fast), broadcast over partitions
  o_norm = o_aug[0:32] * r                   (DVE)
  out = (gamma*Wv)^T^T @ o_norm + x          (PE + DVE residual add)
"""

import os
import sys

sys.path.insert(0, "/opt/trn_rl_repo")

import numpy as np

import concourse.bass as bass
import concourse.mybir as mybir
import concourse.tile as tile
from concourse import bacc
from concourse.bass_utils import run_bass_kernel_spmd

F32 = mybir.dt.float32
F32R = mybir.dt.float32r
BF16 = mybir.dt.bfloat16

P = 128
C = 256          # input channels
CQ = 32          # query channels
D = 24           # spatial extent
NL = 6912        # queries per core (half of 13824)
NF = 13824       # full spatial size
M = 1728         # pooled key/value length
MP = 1792        # m padded to 14*128
NMT = 14         # m tiles of 128
MAXMM = 512      # max matmul free dim (one PSUM bank, fp32)

# q-chunks for the main loop: 8 x 864 (aligned to the 1728-col x tiles)
QN = 864
QCH = [(i * QN, QN) for i in range(8)]
XT = 1728  # x tile width (3 d-slices)


def _subs(n, s=MAXMM):
    """split [0, n) into subchunks of length <= s"""
    return [(o, min(s, n - o)) for o in range(0, n, s)]


def build_nc():
    nc = bacc.Bacc("TRN2", target_bir_lowering=False)

    x_ext = nc.declare_dram_parameter("x", [C, NF], F32R, isOutput=False)
    wf_ext = nc.declare_dram_parameter("wf", [C, CQ], F32R, isOutput=False)
    wg_ext = nc.declare_dram_parameter("wg", [C, CQ], F32R, isOutput=False)
    wh_ext = nc.declare_dram_parameter("wh", [C, CQ], F32R, isOutput=False)
    wv_ext = nc.declare_dram_parameter("wv", [CQ, C], F32R, isOutput=False)
    out_ext = nc.declare_dram_parameter("out", [C, NL], F32, isOutput=True)

    with tile.TileContext(nc) as tc:
        kernel_body(nc, tc, x_ext, wf_ext, wg_ext, wh_ext, wv_ext, out_ext)
    nc.compile()
    return nc


def kernel_body(nc, tc, x_ext, wf_ext, wg_ext, wh_ext, wv_ext, out_ext):
    from contextlib import ExitStack

    with ExitStack() as ctx:
        const = ctx.enter_context(tc.tile_pool(name="const", bufs=1))
        xp = ctx.enter_context(tc.tile_pool(name="xp", bufs=1))
        fgh = ctx.enter_context(tc.tile_pool(name="fgh", bufs=1))
        pool1 = ctx.enter_context(tc.tile_pool(name="pool1", bufs=2))
        pool2 = ctx.enter_context(tc.tile_pool(name="pool2", bufs=2))
        work = ctx.enter_context(tc.tile_pool(name="work", bufs=2))
        nrm = ctx.enter_context(tc.tile_pool(name="nrm", bufs=2))
        # PSUM: sc 2x2 banks + o 1x2 banks + fin 2x1 bank = 8 banks
        ps_sc = ctx.enter_context(tc.tile_pool(name="ps_sc", bufs=2, space="PSUM"))
        ps_o = ctx.enter_context(tc.tile_pool(name="ps_o", bufs=1, space="PSUM"))
        ps_fin = ctx.enter_context(tc.tile_pool(name="ps_fin", bufs=2, space="PSUM"))

        # ---- weights in ----
        wf_sb = const.tile([P, 2 * CQ], F32R)
        wg_sb = const.tile([P, 2 * CQ], F32R)
        wh_sb = const.tile([P, 2 * CQ], F32R)
        wv_sb = const.tile([CQ, C], F32R)
        for w_sb, w_ext in ((wf_sb, wf_ext), (wg_sb, wg_ext), (wh_sb, wh_ext)):
            for k in range(2):
                nc.sync.dma_start(
                    out=w_sb[:, k * CQ:(k + 1) * CQ],
                    in_=w_ext[k * P:(k + 1) * P, :],
                )
        nc.sync.dma_start(out=wv_sb[:, :], in_=wv_ext[:, :])

        eye = const.tile([CQ, CQ], F32)
        from concourse.masks import make_identity
        make_identity(nc, eye[:, :])

        # ---- x in: per (k-chunk, d-slice-of-3) tiles so each consumer
        # depends on exactly one DMA (walrus caps sync waits per matmul) ----
        x_sb = [
            [
                xp.tile([P, XT], F32R, tag=f"x{k}_{ds}", name=f"x_sb{k}_{ds}")
                for ds in range(NF // XT)
            ]
            for k in range(2)
        ]
        for ds in range(NF // XT):
            for k in range(2):
                nc.sync.dma_start(
                    out=x_sb[k][ds][:, :],
                    in_=x_ext[k * P:(k + 1) * P, ds * XT:(ds + 1) * XT],
                )

        def xv(k, c0, cn):
            """view of x columns [c0, c0+cn) -- must lie in one tile"""
            ds, o = divmod(c0, XT)
            assert o + cn <= XT, (c0, cn)
            return x_sb[k][ds][:, o:o + cn]


        # ---- g/h projection + 2x2x2 maxpool ----
        # x spatial layout per core: [d=24, h=24, w=24] flattened; d in two
        # halves of 12 (query half first).  Per d-slice [32, 576]:
        #   proj (PE) -> psum; copy psum->sbuf (ScalarE, idle in prologue);
        #   3 max passes on gpsimd: w-pairs, h-pairs, d-pairs.
        g_sb = fgh.tile([CQ, MP], F32R)
        h_sb = fgh.tile([CQ, MP], F32)
        for t_sb, w_sb in ((g_sb, wg_sb), (h_sb, wh_sb)):
            t2_parts = {}
            for d in range(D):
                # two 288-col halves, each bank-aligned (bank = 512 fp32)
                ps_p = ps_sc.tile([CQ, 2, 512], F32, tag="sc")
                for j in range(2):
                    for k in range(2):
                        nc.tensor.matmul(
                            out=ps_p[:, j, 0:288],
                            lhsT=w_sb[:, k * CQ:(k + 1) * CQ],
                            rhs=xv(k, d * 576 + j * 288, 288),
                            start=(k == 0),
                            stop=(k == 1),
                        )
                pre = pool1.tile([CQ, 576], F32, tag="pre")
                nc.vector.tensor_copy(
                    out=pre[:, :].rearrange("p (j a) -> p j a", j=2),
                    in_=ps_p[:, :, 0:288],
                )
                # pass1: w-pairs [h=24, w=24] -> [24, 12]
                t1 = pool1.tile([CQ, 288], F32, tag="t1")
                prev = pre[:, :].rearrange("p (a b) -> p a b", b=2)
                nc.vector.tensor_tensor(
                    out=t1[:, :],
                    in0=prev[:, :, 0],
                    in1=prev[:, :, 1],
                    op=mybir.AluOpType.max,
                )
                # pass2: h-pairs [24, 12] -> [12, 12]
                t2 = pool2.tile([CQ, 144], F32, tag="t2")
                t1v = t1[:, :].rearrange("p (h2 q w) -> p h2 q w", h2=12, q=2)
                nc.vector.tensor_tensor(
                    out=t2[:, :].rearrange("p (h2 w) -> p h2 w", h2=12),
                    in0=t1v[:, :, 0, :],
                    in1=t1v[:, :, 1, :],
                    op=mybir.AluOpType.max,
                )
                t2_parts[d] = t2
                # pass3: d-pairs
                if d % 2 == 1:
                    nc.vector.tensor_tensor(
                        out=t_sb[:, (d // 2) * 144:(d // 2) * 144 + 144],
                        in0=t2_parts[d - 1][:, :],
                        in1=t2[:, :],
                        op=mybir.AluOpType.max,
                    )
            # pad m 1728:1792 with zeros (exp(0)=1 but excluded from sums).
            # memset can't write float32r; multiply real data by 0 instead.
            nc.vector.tensor_scalar_mul(
                out=t_sb[:, M:MP], in0=t_sb[:, M - (MP - M):M], scalar1=0.0
            )

        # ---- hT_aug [128, 14*64] bf16: transposed h, a ones column at 32,
        # zero pad to 64 (matmul stationary columns must be 32-aligned) ----
        MTW = 2 * CQ  # 64 cols per m-tile
        hT = fgh.tile([P, NMT * MTW], BF16)
        nc.vector.memset(hT[:, :], 0.0)
        for mt in range(NMT):
            ps_t = ps_fin.tile([P, CQ], F32, tag="fin")
            nc.tensor.transpose(
                out=ps_t[:, :],
                in_=h_sb[:, mt * P:(mt + 1) * P],
                identity=eye[:, :],
            )
            c0 = mt * MTW
            nc.vector.tensor_copy(out=hT[:, c0:c0 + CQ], in_=ps_t[:, :])
            if mt < NMT - 1:
                nc.vector.memset(hT[:, c0 + CQ:c0 + CQ + 1], 1.0)
            else:
                # last tile: only m 1664:1728 are real
                nc.vector.memset(hT[0:M - 13 * P, c0 + CQ:c0 + CQ + 1], 1.0)

        # ---- main loop over q-chunks ----
        # persistent r tiles (stream_shuffle reads all 32 partitions; memset
        # once so partitions 1:32 are initialized)
        r_ts = [nrm.tile([CQ, QN], F32, tag=f"r{i}", name=f"r_t{i}") for i in range(2)]
        for r_t in r_ts:
            nc.vector.memset(r_t[:, :], 0.0)
        for qi, (q0, qn) in enumerate(QCH):
            # f projection for this q-chunk only (saves 27.6KB of SBUF)
            ps_fc = ps_sc.tile([CQ, 1024], F32, tag="sc")
            for s0, sn in _subs(qn):
                for k in range(2):
                    nc.tensor.matmul(
                        out=ps_fc[:, s0:s0 + sn],
                        lhsT=wf_sb[:, k * CQ:(k + 1) * CQ],
                        rhs=xv(k, q0 + s0, sn),
                        start=(k == 0),
                        stop=(k == 1),
                    )
            f_c = work.tile([CQ, QN], F32R, tag="fc")
            nc.vector.tensor_copy(out=f_c[:, 0:qn], in_=ps_fc[:, 0:qn])
            ps_oa = ps_o.tile([2 * CQ, QN], F32, tag="o")

            # software-pipelined m-loop: issue scores(mt+1) before o(mt) so
            # the in-order PE stream never stalls waiting for exp(mt)
            def emit_scores(mt):
                ps_s = ps_sc.tile([P, 1024], F32, tag="sc", name=f"ps_s_{qi}_{mt}")
                for s0, sn in _subs(qn):
                    nc.tensor.matmul(
                        out=ps_s[:, s0:s0 + sn],
                        lhsT=g_sb[:, mt * P:(mt + 1) * P],
                        rhs=f_c[:, s0:s0 + sn],
                        start=True,
                        stop=True,
                    )
                est = work.tile([P, QN], BF16, tag="est", name=f"est_{qi}_{mt}", bufs=3)
                nc.scalar.activation(
                    out=est[:, 0:qn],
                    in_=ps_s[:, 0:qn],
                    func=mybir.ActivationFunctionType.Exp,
                )
                return est

            def emit_o(mt, est):
                for s0, sn in _subs(qn):
                    nc.tensor.matmul(
                        out=ps_oa[:, s0:s0 + sn],
                        lhsT=hT[:, mt * MTW:(mt + 1) * MTW],
                        rhs=est[:, s0:s0 + sn],
                        start=(mt == 0),
                        stop=(mt == NMT - 1),
                    )

            prev = emit_scores(0)
            for mt in range(1, NMT):
                cur = emit_scores(mt)
                emit_o(mt - 1, prev)
                prev = cur
            emit_o(NMT - 1, prev)
            # softmax denominators -> broadcast reciprocal.  DVE lanes are
            # partition-locked, so stage the sums row: copy psum[32]->sbuf[32]
            # then DMA it to partition 0, then everything stays aligned.
            srow = nrm.tile([CQ + 1, QN], F32, tag="srow")
            nc.vector.tensor_copy(
                out=srow[CQ:CQ + 1, 0:qn], in_=ps_oa[CQ:CQ + 1, 0:qn]
            )
            nc.sync.dma_start(out=srow[0:1, 0:qn], in_=srow[CQ:CQ + 1, 0:qn])
            r_t = r_ts[qi % 2]
            nc.vector.reciprocal_approx_fast(
                out=r_t[0:1, 0:qn], in_=srow[0:1, 0:qn]
            )
            nc.vector.stream_shuffle(
                out=r_t[:, 0:qn], in_=r_t[:, 0:qn], mask=[0] * 32
            )
            o_n = nrm.tile([CQ, QN], F32R, tag="on")
            nc.vector.tensor_tensor(
                out=o_n[:, 0:qn],
                in0=ps_oa[0:CQ, 0:qn],
                in1=r_t[:, 0:qn],
                op=mybir.AluOpType.mult,
            )
            # final projection + residual + store
            for cc in range(2):
                for s0, sn in _subs(qn):
                    ps_f2 = ps_fin.tile([P, MAXMM], F32, tag="fin")
                    nc.tensor.matmul(
                        out=ps_f2[:, 0:sn],
                        lhsT=wv_sb[:, cc * P:(cc + 1) * P],
                        rhs=o_n[:, s0:s0 + sn],
                        start=True,
                        stop=True,
                    )
                    o_sb = work.tile([P, MAXMM], F32, tag="osb")
                    nc.vector.tensor_tensor(
                        out=o_sb[:, 0:sn],
                        in0=ps_f2[:, 0:sn],
                        in1=xv(cc, q0 + s0, sn),
                        op=mybir.AluOpType.add,
                    )
                    nc.sync.dma_start(
                        out=out_ext[cc * P:(cc + 1) * P, q0 + s0:q0 + s0 + sn],
                        in_=o_sb[:, 0:sn],
                    )


_NC_CACHE = None


def _get_nc():
    global _NC_CACHE
    if _NC_CACHE is None:
        _NC_CACHE = build_nc()
    return _NC_CACHE


def _prep_in_maps(x, Wf, Wg, Wh, Wv, gamma):
    """Build the 8 per-core input maps (host-side shard + permute)."""
    B = x.shape[0]
    x = np.asarray(x, dtype=np.float32).reshape(B, C, D, D * D)
    wvg = (np.float32(gamma) * np.asarray(Wv, dtype=np.float32)).T.copy()
    wfT = np.ascontiguousarray(np.asarray(Wf, np.float32).T)
    wgT = np.ascontiguousarray(np.asarray(Wg, np.float32).T)
    whT = np.ascontiguousarray(np.asarray(Wh, np.float32).T)
    in_maps = []
    for c in range(8):
        b, s = divmod(c, 2)
        xb = x[b]
        if s == 0:
            xp = xb
        else:
            xp = np.concatenate([xb[:, 12:24], xb[:, 0:12]], axis=1)
        in_maps.append({
            "x": np.ascontiguousarray(xp.reshape(C, NF)),
            "wf": wfT, "wg": wgT, "wh": whT, "wv": wvg,
        })
    return in_maps


def run(inputs, trace=False, **kw):
    nc = _get_nc()
    in_maps = _prep_in_maps(**inputs)
    res = run_bass_kernel_spmd(
        nc, in_maps, core_ids=list(range(8)), trace=trace, **kw
    )
    x = np.asarray(inputs["x"], dtype=np.float32)
    B = x.shape[0]
    out = np.empty((B, C, D, D, D), dtype=np.float32)
    for c in range(8):
        b, s = divmod(c, 2)
        out[b, :, 12 * s:12 * (s + 1)] = res.results[c]["out"].reshape(C, 12, D, D)
    return out, res


def kernel(**inputs):
    return run(inputs)[0]
